# revision 19
# baseline (speedup 1.0000x reference)
"""CalderaLinear Trainium2 kernel, v5 (fp8 DoubleRow q-path, pipelined).

Computes out = x @ dequant(q).T + (x @ dequant(r).T) @ dequant(l).T + bias
with groupwise (group=128) dequantization, distributed over 8 NeuronCores
by sharding tokens (batch*seq) 8 ways and replicating the weights.

Device work: dequant (scale replication via stride-0 broadcast DMA + a
tensor multiply split across the vector and gpsimd engines), the q-path
GEMM in fp8 DoubleRow mode (2 contraction groups per instruction), the
low-rank path in bf16 (quantization noise there rides the coherent mean
components of r/l and is heavily amplified, so it stays 16-bit), bias add
fused into the PSUM drain. Q slabs are software-pipelined 2 deep.

Host work (layout/cast only): shard tokens, transpose weights/activations
to contraction-major layouts, lossless int->fp8/bf16 casts of quantized
values (0..15 exact in e4m3 and bf16), fp32->fp8/bf16 casts of x, scale
transposes/casts, final concat of per-core output shards.
"""

import os
import sys

import numpy as np
import ml_dtypes

for _p in ("/opt/trn_rl_repo",):
    if _p not in sys.path and os.path.isdir(_p):
        sys.path.insert(0, _p)

import concourse.bass as bass
import concourse.mybir as mybir
import concourse.tile as tile
from concourse import bacc
from concourse.bass_utils import run_bass_kernel_spmd

BF16 = mybir.dt.bfloat16
F32 = mybir.dt.float32
FP8 = mybir.dt.float8e4
DR = mybir.MatmulPerfMode.DoubleRow

P = 128  # partitions / quant group size
N_CORES = 8

# Full problem shape (hardcoded per contest contract).
B, S, D_IN, D_OUT, RANK = 4, 2048, 4096, 4096, 256
N_TOK = B * S  # 8192
T_SH = N_TOK // N_CORES  # 1024 tokens per core


def caldera_v5(tc, out, xTb, qT8, qsS, rT, rsF, lT, lsF, bias_, obw=512):
    """One core's program. DRAM layouts (K=in, O=out, R=rank, T=tokens):
    xTb  [P, K/P, T]        bf16  x.T chunks: xTb[p,g,t] = x[t, g*P+p]
                                  (fp8 copy for the q path is cast on-device)
    qT8  [NOB, P, K/P, OBW] fp8   q.T slab-major: [ob,p,g,oo] = q[ob*OBW+oo, g*P+p]
    qsS  [NOB, 1, K/P, OBW] fp8   q_scales slab-major: [ob,0,g,oo] = qs[ob*OBW+oo, g]
    rT   [P, K/P, R]        bf16  r.T chunks
    rsF  [1, K/P, R]        bf16  r_scales.T (flat, partition 0)
    lT   [P, R/P, O]        bf16  l.T chunks
    lsF  [1, R/P, O]        bf16  l_scales.T (flat, partition 0)
    bias_ [1, O]            bf16
    out  [T, O]             f32
    """
    nc = tc.nc
    T = xTb.shape[2]
    KC = xTb.shape[1]          # contraction groups (32)
    RC = lT.shape[1]           # rank groups (2)
    R = rT.shape[2]
    O = lT.shape[2]
    OBW = obw
    NOB = O // OBW             # output slabs (8)
    TS = T // P                # token tiles (8)
    NGP = KC // 2              # DoubleRow group pairs (16)
    KH = KC // 2               # half the groups (DMA split)
    CH = max(1, KC // 4)       # dequant chunk (2 on DVE, 2 on gpsimd)

    with tc.tile_pool(name="const", bufs=1) as constp, \
         tc.tile_pool(name="qslp", bufs=3) as qslp, \
         tc.tile_pool(name="repp", bufs=2) as repp, \
         tc.tile_pool(name="xbfp", bufs=2) as xbfp, \
         tc.tile_pool(name="outp", bufs=4) as outp, \
         tc.tile_pool(name="psm", bufs=8, space="PSUM") as psm:

        # ---- resident tensors ----
        x8 = constp.tile([P, KC, T], FP8)
        rt = constp.tile([P, KC, R], BF16)
        lt = constp.tile([P, RC, O], BF16)
        xrT = constp.tile([P, RC, T], BF16)
        bias_bc = constp.tile([P, O], BF16)

        def slab_load(ob):
            """Prefetch + dequantize q slab ob. Returns the fp8 tile."""
            qsl = qslp.tile([P, KC, OBW], FP8, tag="q", name=f"qsl_{ob}")
            rep = repp.tile([P, KC, OBW], FP8, tag="rep", name=f"rep_{ob}")
            for h in range(2):
                nc.sync.dma_start(out=qsl[:, h * KH:(h + 1) * KH, :],
                                  in_=qT8[ob][:, h * KH:(h + 1) * KH, :])
                nc.sync.dma_start(
                    out=rep[:, h * KH:(h + 1) * KH, :],
                    in_=qsS[ob][0:1, None, h * KH:(h + 1) * KH, :]
                    .broadcast_to([1, P, KH, OBW]))
            for ci, eng in ((0, nc.vector), (1, nc.gpsimd),
                            (2, nc.vector), (3, nc.gpsimd)):
                sl = slice(ci * CH, (ci + 1) * CH)
                eng.tensor_tensor(out=qsl[:, sl, :], in0=qsl[:, sl, :],
                                  in1=rep[:, sl, :],
                                  op=mybir.AluOpType.mult)
            return qsl

        # ---- prologue (scoped pool; space reclaimed for main loop) ----
        # The prologue is HBM-bound: only rt/xTb/slab0/lt reads matter.
        # x8 (fp8 x for the q path) is cast on-device from the bf16 x
        # chunks by the otherwise-idle scalar engine, saving 4.2MB of DMA.
        with tc.tile_pool(name="pro", bufs=1) as prop:
            # r scales replicated across partitions (DMA broadcast from DRAM)
            reps_r = prop.tile([P, KC, R], BF16, tag="reps")
            nc.sync.dma_start(
                out=reps_r[:],
                in_=rsF[0:1, None, :, :].broadcast_to([1, P, KC, R]))
            nc.sync.dma_start(out=rt[:], in_=rT[:])

            # first two x chunks for the xr phase
            TH = min(512, T)
            NTH = T // TH
            NRH = R // P
            GBLK = min(4, KC)
            NGB = KC // GBLK

            def xc_load(gb):
                xc = xbfp.tile([P, GBLK, T], BF16, tag="xbf", name=f"xc_{gb}")
                nc.sync.dma_start(out=xc[:],
                                  in_=xTb[:, gb * GBLK:(gb + 1) * GBLK, :])
                # fp8 copy for the main-loop q path (scalar engine cast)
                nc.scalar.copy(x8[:, gb * GBLK:(gb + 1) * GBLK, :], xc[:])
                return xc

            xcs = {gb: xc_load(gb) for gb in range(min(2, NGB))}

            # r dequant in one DVE op
            nc.vector.tensor_tensor(out=rt[:], in0=rt[:], in1=reps_r[:],
                                    op=mybir.AluOpType.mult)

            # slab 0 (needed at main-loop start)
            slabs = [slab_load(0)]

            # xr.T = (x @ r_deq.T).T via r_chunk @ x.T, x streamed bf16
            pxr = [[psm.tile([P, TH], F32, tag="mm", name=f"pxr_{rh}_{th}")
                    for th in range(NTH)]
                   for rh in range(NRH)]
            for gb in range(NGB):
                if gb not in xcs:
                    xcs[gb] = xc_load(gb)
                xc = xcs[gb]
                for gg in range(GBLK):
                    g = gb * GBLK + gg
                    for rh in range(NRH):
                        for th in range(NTH):
                            nc.tensor.matmul(
                                pxr[rh][th][:],
                                lhsT=rt[:, g, rh * P:(rh + 1) * P],
                                rhs=xc[:, gg, th * TH:(th + 1) * TH],
                                start=(g == 0), stop=(g == KC - 1),
                            )
            # l / bias / slab 1 (needed a few us into the main loop; queued
            # behind the x chunks so they don't starve the xr phase)
            nc.sync.dma_start(out=lt[:], in_=lT[:])
            reps_l = prop.tile([P, RC, O], BF16, tag="reps", name="reps_l")
            nc.sync.dma_start(
                out=reps_l[:],
                in_=lsF[0:1, None, :, :].broadcast_to([1, P, RC, O]))
            # l dequant split across vector/gpsimd so neither queue blocks
            for cg, eng in ((0, nc.vector), (1, nc.gpsimd)):
                eng.tensor_tensor(out=lt[:, cg, :], in0=lt[:, cg, :],
                                  in1=reps_l[:, cg, :],
                                  op=mybir.AluOpType.mult)
            nc.sync.dma_start(out=bias_bc[:],
                              in_=bias_[0:1, None, :].broadcast_to([1, P, O]))
            slabs.append(slab_load(1))

            for rh in range(NRH):
                for th in range(NTH):
                    nc.scalar.copy(xrT[:, rh, th * TH:(th + 1) * TH],
                                   pxr[rh][th][:])

        # ---- main loop over output slabs (2-deep slab pipeline) ----
        for ob in range(NOB):
            qsl = slabs[ob]
            for t in range(TS):
                ps = psm.tile([P, OBW], F32, tag="mm")
                for gp in range(NGP):
                    nc.tensor.matmul(
                        ps[:],
                        lhsT=x8[:, 2 * gp:2 * gp + 2, t * P:(t + 1) * P],
                        rhs=qsl[:, 2 * gp:2 * gp + 2, :],
                        start=(gp == 0), stop=False, perf_mode=DR,
                    )
                for c in range(RC):
                    nc.tensor.matmul(
                        ps[:],
                        lhsT=xrT[:, c, t * P:(t + 1) * P],
                        rhs=lt[:, c, ob * OBW:(ob + 1) * OBW],
                        start=False, stop=(c == RC - 1),
                    )
                osb = outp.tile([P, OBW], F32)
                nc.vector.tensor_tensor(
                    out=osb[:], in0=ps[:],
                    in1=bias_bc[:, ob * OBW:(ob + 1) * OBW],
                    op=mybir.AluOpType.add)
                nc.sync.dma_start(
                    out=out[t * P:(t + 1) * P, ob * OBW:(ob + 1) * OBW],
                    in_=osb[:],
                )
                if t == 0 and ob + 2 < NOB:
                    slabs.append(slab_load(ob + 2))


def build_nc(T=T_SH, O=D_OUT, K=D_IN, R=RANK, obw=512):
    nc = bacc.Bacc("TRN2", target_bir_lowering=False, debug=False)
    KC = K // P
    RC = R // P
    NOB = O // obw
    xTb = nc.dram_tensor("xTb", [P, KC, T], BF16, kind="ExternalInput").ap()
    qT8 = nc.dram_tensor("qT8", [NOB, P, KC, obw], FP8, kind="ExternalInput").ap()
    qsS = nc.dram_tensor("qsS", [NOB, 1, KC, obw], FP8, kind="ExternalInput").ap()
    rT = nc.dram_tensor("rT", [P, KC, R], BF16, kind="ExternalInput").ap()
    rsF = nc.dram_tensor("rsF", [1, KC, R], BF16, kind="ExternalInput").ap()
    lT = nc.dram_tensor("lT", [P, RC, O], BF16, kind="ExternalInput").ap()
    lsF = nc.dram_tensor("lsF", [1, RC, O], BF16, kind="ExternalInput").ap()
    bias_ = nc.dram_tensor("bias", [1, O], BF16, kind="ExternalInput").ap()
    out = nc.dram_tensor("out", [T, O], F32, kind="ExternalOutput").ap()
    with tile.TileContext(nc) as tc:
        caldera_v5(tc, out, xTb, qT8, qsS, rT, rsF, lT, lsF, bias_,
                   obw=obw)
    nc.compile()
    return nc


def _chunked_T(a, part=P):
    """[N, K] -> [P, K//P, N] with out[p, g, n] = a[n, g*P+p]."""
    n, k = a.shape
    return np.ascontiguousarray(a.T.reshape(k // part, part, n).transpose(1, 0, 2))


def make_in_maps(x, q_values, q_scales, l_values, l_scales, r_values, r_scales,
                 bias, obw=512):
    bf16 = ml_dtypes.bfloat16
    f8 = ml_dtypes.float8_e4m3
    KC = D_IN // P
    NOB = D_OUT // obw

    xf = np.asarray(x, dtype=np.float32).reshape(N_TOK, D_IN)
    qv = np.asarray(q_values)
    # q.T slab-major fp8: [NOB, P, KC, OBW]
    qT = np.ascontiguousarray(
        qv.T.reshape(KC, P, NOB, obw).transpose(2, 1, 0, 3)).astype(f8)
    # q scales slab-major on partition 0: [NOB, 1, KC, OBW]
    qsS = np.ascontiguousarray(
        np.asarray(q_scales, dtype=np.float32).T  # [KC, O]
        .reshape(KC, NOB, obw).transpose(1, 0, 2)[:, None]).astype(f8)
    rT = _chunked_T(np.asarray(r_values).astype(np.float32)).astype(bf16)
    rsF = np.ascontiguousarray(
        np.asarray(r_scales, dtype=np.float32).T[None]).astype(bf16)
    lT = _chunked_T(np.asarray(l_values).astype(np.float32)).astype(bf16)
    lsF = np.ascontiguousarray(
        np.asarray(l_scales, dtype=np.float32).T[None]).astype(bf16)
    b = np.asarray(bias, dtype=np.float32).reshape(1, D_OUT).astype(bf16)

    in_maps = []
    for i in range(N_CORES):
        xs = xf[i * T_SH:(i + 1) * T_SH]
        in_maps.append({
            "xTb": _chunked_T(xs).astype(bf16),
            "qT8": qT, "qsS": qsS,
            "rT": rT, "rsF": rsF, "lT": lT, "lsF": lsF,
            "bias": b,
        })
    return in_maps


_NC_CACHE = {}


def _get_nc():
    if "nc" not in _NC_CACHE:
        _NC_CACHE["nc"] = build_nc()
    return _NC_CACHE["nc"]


def run(inputs, trace=False, tmpdir=None):
    nc = _get_nc()
    in_maps = make_in_maps(**inputs)
    res = run_bass_kernel_spmd(
        nc, in_maps, list(range(N_CORES)), trace=trace, tmpdir=tmpdir
    )
    shards = [np.asarray(res.results[i]["out"]) for i in range(N_CORES)]
    full = np.concatenate(shards, axis=0).reshape(B, S, D_OUT)
    return full.astype(np.float32), res


def kernel(**inputs) -> np.ndarray:
    out, _ = run(inputs, trace=False)
    return out


# revision 20
# speedup vs baseline: 1.1556x; 1.1556x over previous
"""CalderaLinear Trainium2 kernel, v5 (fp8 DoubleRow q-path, pipelined).

Computes out = x @ dequant(q).T + (x @ dequant(r).T) @ dequant(l).T + bias
with groupwise (group=128) dequantization, distributed over 8 NeuronCores
by sharding tokens (batch*seq) 8 ways and replicating the weights.

Device work: dequant (scale replication via stride-0 broadcast DMA + a
tensor multiply split across the vector and gpsimd engines), the q-path
GEMM in fp8 DoubleRow mode (2 contraction groups per instruction), the
low-rank path in bf16 (quantization noise there rides the coherent mean
components of r/l and is heavily amplified, so it stays 16-bit), bias add
fused into the PSUM drain. Q slabs are software-pipelined 2 deep.

Host work (layout/cast only): shard tokens, transpose weights/activations
to contraction-major layouts, lossless int->fp8/bf16 casts of quantized
values (0..15 exact in e4m3 and bf16), fp32->fp8/bf16 casts of x, scale
transposes/casts, final concat of per-core output shards.
"""

import os
import sys

import numpy as np
import ml_dtypes

for _p in ("/opt/trn_rl_repo",):
    if _p not in sys.path and os.path.isdir(_p):
        sys.path.insert(0, _p)

import concourse.bass as bass
import concourse.mybir as mybir
import concourse.tile as tile
from concourse import bacc
from concourse.bass_utils import run_bass_kernel_spmd

BF16 = mybir.dt.bfloat16
F32 = mybir.dt.float32
FP8 = mybir.dt.float8e4
DR = mybir.MatmulPerfMode.DoubleRow

P = 128  # partitions / quant group size
N_CORES = 8

# Full problem shape (hardcoded per contest contract).
B, S, D_IN, D_OUT, RANK = 4, 2048, 4096, 4096, 256
N_TOK = B * S  # 8192
T_SH = N_TOK // N_CORES  # 1024 tokens per core


def caldera_v5(tc, out, xTb, qT8, qsS, rT, rsF, lT, lsF, bias_, obw=512):
    """One core's program. DRAM layouts (K=in, O=out, R=rank, T=tokens):
    xTb  [P, K/P, T]        bf16  x.T chunks: xTb[p,g,t] = x[t, g*P+p]
                                  (fp8 copy for the q path is cast on-device)
    qT8  [NOB, P, K/P, OBW] fp8   q.T slab-major: [ob,p,g,oo] = q[ob*OBW+oo, g*P+p]
    qsS  [NOB, 1, K/P, OBW] fp8   q_scales slab-major: [ob,0,g,oo] = qs[ob*OBW+oo, g]
    rT   [P, K/P, R]        bf16  r.T chunks
    rsF  [1, K/P, R]        bf16  r_scales.T (flat, partition 0)
    lT   [P, R/P, O]        bf16  l.T chunks
    lsF  [1, R/P, O]        bf16  l_scales.T (flat, partition 0)
    bias_ [1, O]            bf16
    out  [T, O]             f32
    """
    nc = tc.nc
    T = xTb.shape[2]
    KC = xTb.shape[1]          # contraction groups (32)
    RC = lT.shape[1]           # rank groups (2)
    R = rT.shape[2]
    O = lT.shape[2]
    OBW = obw
    NOB = O // OBW             # output slabs (8)
    TS = T // P                # token tiles (8)
    NGP = KC // 2              # DoubleRow group pairs (16)
    KH = KC // 2               # half the groups (DMA split)
    CH = max(1, KC // 4)       # dequant chunk (2 on DVE, 2 on gpsimd)

    with tc.tile_pool(name="const", bufs=1) as constp, \
         tc.tile_pool(name="qslp", bufs=3) as qslp, \
         tc.tile_pool(name="repp", bufs=2) as repp, \
         tc.tile_pool(name="xbfp", bufs=2) as xbfp, \
         tc.tile_pool(name="outp", bufs=4) as outp, \
         tc.tile_pool(name="psm", bufs=8, space="PSUM") as psm:

        # ---- resident tensors ----
        x8 = constp.tile([P, KC, T], FP8)
        rt = constp.tile([P, KC, R], BF16)
        lt = constp.tile([P, RC, O], BF16)
        xrT = constp.tile([P, RC, T], BF16)
        bias_bc = constp.tile([P, O], BF16)

        def slab_load(ob):
            """Prefetch + dequantize q slab ob. Returns the fp8 tile."""
            qsl = qslp.tile([P, KC, OBW], FP8, tag="q", name=f"qsl_{ob}")
            rep = repp.tile([P, KC, OBW], FP8, tag="rep", name=f"rep_{ob}")
            for h in range(2):
                nc.sync.dma_start(out=qsl[:, h * KH:(h + 1) * KH, :],
                                  in_=qT8[ob][:, h * KH:(h + 1) * KH, :])
                nc.sync.dma_start(
                    out=rep[:, h * KH:(h + 1) * KH, :],
                    in_=qsS[ob][0:1, None, h * KH:(h + 1) * KH, :]
                    .broadcast_to([1, P, KH, OBW]))
            for ci, eng in ((0, nc.vector), (1, nc.gpsimd),
                            (2, nc.vector), (3, nc.gpsimd)):
                sl = slice(ci * CH, (ci + 1) * CH)
                eng.tensor_tensor(out=qsl[:, sl, :], in0=qsl[:, sl, :],
                                  in1=rep[:, sl, :],
                                  op=mybir.AluOpType.mult)
            return qsl

        # ---- prologue (scoped pool; space reclaimed for main loop) ----
        # The prologue is HBM-bound: only rt/xTb/slab0/lt reads matter.
        # x8 (fp8 x for the q path) is cast on-device from the bf16 x
        # chunks by the otherwise-idle scalar engine, saving 4.2MB of DMA.
        with tc.tile_pool(name="pro", bufs=1) as prop:
            # r scales replicated across partitions (DMA broadcast from DRAM)
            reps_r = prop.tile([P, KC, R], BF16, tag="reps")
            nc.sync.dma_start(
                out=reps_r[:],
                in_=rsF[0:1, None, :, :].broadcast_to([1, P, KC, R]))
            nc.sync.dma_start(out=rt[:], in_=rT[:])

            # first two x chunks for the xr phase
            TH = min(512, T)
            NTH = T // TH
            NRH = R // P
            GBLK = min(4, KC)
            NGB = KC // GBLK

            def xc_load(gb):
                xc = xbfp.tile([P, GBLK, T], BF16, tag="xbf", name=f"xc_{gb}")
                nc.sync.dma_start(out=xc[:],
                                  in_=xTb[:, gb * GBLK:(gb + 1) * GBLK, :])
                # fp8 copy for the main-loop q path (scalar engine cast)
                nc.scalar.copy(x8[:, gb * GBLK:(gb + 1) * GBLK, :], xc[:])
                return xc

            xcs = {gb: xc_load(gb) for gb in range(min(2, NGB))}

            # r dequant in one DVE op
            nc.vector.tensor_tensor(out=rt[:], in0=rt[:], in1=reps_r[:],
                                    op=mybir.AluOpType.mult)

            # slab 0 (needed at main-loop start), l/bias (a few us in)
            slabs = [slab_load(0)]
            nc.sync.dma_start(out=lt[:], in_=lT[:])
            reps_l = prop.tile([P, RC, O], BF16, tag="reps", name="reps_l")
            nc.sync.dma_start(
                out=reps_l[:],
                in_=lsF[0:1, None, :, :].broadcast_to([1, P, RC, O]))
            # l dequant split across vector/gpsimd so neither queue blocks
            for cg, eng in ((0, nc.vector), (1, nc.gpsimd)):
                eng.tensor_tensor(out=lt[:, cg, :], in0=lt[:, cg, :],
                                  in1=reps_l[:, cg, :],
                                  op=mybir.AluOpType.mult)
            nc.sync.dma_start(out=bias_bc[:],
                              in_=bias_[0:1, None, :].broadcast_to([1, P, O]))

            # xr.T = (x @ r_deq.T).T via r_chunk @ x.T, x streamed bf16
            pxr = [[psm.tile([P, TH], F32, tag="mm", name=f"pxr_{rh}_{th}")
                    for th in range(NTH)]
                   for rh in range(NRH)]
            for gb in range(NGB):
                if gb not in xcs:
                    xcs[gb] = xc_load(gb)
                xc = xcs[gb]
                for gg in range(GBLK):
                    g = gb * GBLK + gg
                    for rh in range(NRH):
                        for th in range(NTH):
                            nc.tensor.matmul(
                                pxr[rh][th][:],
                                lhsT=rt[:, g, rh * P:(rh + 1) * P],
                                rhs=xc[:, gg, th * TH:(th + 1) * TH],
                                start=(g == 0), stop=(g == KC - 1),
                            )
            # slab 1 (needed ~30us into the main loop; queued behind the
            # x chunks so it doesn't starve the xr phase)
            slabs.append(slab_load(1))

            for rh in range(NRH):
                for th in range(NTH):
                    nc.scalar.copy(xrT[:, rh, th * TH:(th + 1) * TH],
                                   pxr[rh][th][:])

        # ---- main loop over output slabs (2-deep slab pipeline) ----
        for ob in range(NOB):
            qsl = slabs[ob]
            for t in range(TS):
                ps = psm.tile([P, OBW], F32, tag="mm")
                for gp in range(NGP):
                    nc.tensor.matmul(
                        ps[:],
                        lhsT=x8[:, 2 * gp:2 * gp + 2, t * P:(t + 1) * P],
                        rhs=qsl[:, 2 * gp:2 * gp + 2, :],
                        start=(gp == 0), stop=False, perf_mode=DR,
                    )
                for c in range(RC):
                    nc.tensor.matmul(
                        ps[:],
                        lhsT=xrT[:, c, t * P:(t + 1) * P],
                        rhs=lt[:, c, ob * OBW:(ob + 1) * OBW],
                        start=False, stop=(c == RC - 1),
                    )
                osb = outp.tile([P, OBW], F32)
                nc.vector.tensor_tensor(
                    out=osb[:], in0=ps[:],
                    in1=bias_bc[:, ob * OBW:(ob + 1) * OBW],
                    op=mybir.AluOpType.add)
                nc.sync.dma_start(
                    out=out[t * P:(t + 1) * P, ob * OBW:(ob + 1) * OBW],
                    in_=osb[:],
                )
                if t == 0 and ob + 2 < NOB:
                    slabs.append(slab_load(ob + 2))


def build_nc(T=T_SH, O=D_OUT, K=D_IN, R=RANK, obw=512):
    nc = bacc.Bacc("TRN2", target_bir_lowering=False, debug=False)
    KC = K // P
    RC = R // P
    NOB = O // obw
    xTb = nc.dram_tensor("xTb", [P, KC, T], BF16, kind="ExternalInput").ap()
    qT8 = nc.dram_tensor("qT8", [NOB, P, KC, obw], FP8, kind="ExternalInput").ap()
    qsS = nc.dram_tensor("qsS", [NOB, 1, KC, obw], FP8, kind="ExternalInput").ap()
    rT = nc.dram_tensor("rT", [P, KC, R], BF16, kind="ExternalInput").ap()
    rsF = nc.dram_tensor("rsF", [1, KC, R], BF16, kind="ExternalInput").ap()
    lT = nc.dram_tensor("lT", [P, RC, O], BF16, kind="ExternalInput").ap()
    lsF = nc.dram_tensor("lsF", [1, RC, O], BF16, kind="ExternalInput").ap()
    bias_ = nc.dram_tensor("bias", [1, O], BF16, kind="ExternalInput").ap()
    out = nc.dram_tensor("out", [T, O], F32, kind="ExternalOutput").ap()
    with tile.TileContext(nc) as tc:
        caldera_v5(tc, out, xTb, qT8, qsS, rT, rsF, lT, lsF, bias_,
                   obw=obw)
    nc.compile()
    return nc


def _chunked_T(a, part=P):
    """[N, K] -> [P, K//P, N] with out[p, g, n] = a[n, g*P+p]."""
    n, k = a.shape
    return np.ascontiguousarray(a.T.reshape(k // part, part, n).transpose(1, 0, 2))


def make_in_maps(x, q_values, q_scales, l_values, l_scales, r_values, r_scales,
                 bias, obw=512):
    bf16 = ml_dtypes.bfloat16
    f8 = ml_dtypes.float8_e4m3
    KC = D_IN // P
    NOB = D_OUT // obw

    xf = np.asarray(x, dtype=np.float32).reshape(N_TOK, D_IN)
    qv = np.asarray(q_values)
    # q.T slab-major fp8: [NOB, P, KC, OBW]
    qT = np.ascontiguousarray(
        qv.T.reshape(KC, P, NOB, obw).transpose(2, 1, 0, 3)).astype(f8)
    # q scales slab-major on partition 0: [NOB, 1, KC, OBW]
    qsS = np.ascontiguousarray(
        np.asarray(q_scales, dtype=np.float32).T  # [KC, O]
        .reshape(KC, NOB, obw).transpose(1, 0, 2)[:, None]).astype(f8)
    rT = _chunked_T(np.asarray(r_values).astype(np.float32)).astype(bf16)
    rsF = np.ascontiguousarray(
        np.asarray(r_scales, dtype=np.float32).T[None]).astype(bf16)
    lT = _chunked_T(np.asarray(l_values).astype(np.float32)).astype(bf16)
    lsF = np.ascontiguousarray(
        np.asarray(l_scales, dtype=np.float32).T[None]).astype(bf16)
    b = np.asarray(bias, dtype=np.float32).reshape(1, D_OUT).astype(bf16)

    in_maps = []
    for i in range(N_CORES):
        xs = xf[i * T_SH:(i + 1) * T_SH]
        in_maps.append({
            "xTb": _chunked_T(xs).astype(bf16),
            "qT8": qT, "qsS": qsS,
            "rT": rT, "rsF": rsF, "lT": lT, "lsF": lsF,
            "bias": b,
        })
    return in_maps


_NC_CACHE = {}


def _get_nc():
    if "nc" not in _NC_CACHE:
        _NC_CACHE["nc"] = build_nc()
    return _NC_CACHE["nc"]


def run(inputs, trace=False, tmpdir=None):
    nc = _get_nc()
    in_maps = make_in_maps(**inputs)
    res = run_bass_kernel_spmd(
        nc, in_maps, list(range(N_CORES)), trace=trace, tmpdir=tmpdir
    )
    shards = [np.asarray(res.results[i]["out"]) for i in range(N_CORES)]
    full = np.concatenate(shards, axis=0).reshape(B, S, D_OUT)
    return full.astype(np.float32), res


def kernel(**inputs) -> np.ndarray:
    out, _ = run(inputs, trace=False)
    return out
